# revision 1
# baseline (speedup 1.0000x reference)
"""Causal self-attention (B=4, T=2048, C=1024, H=16, D=64) on 8 TRN2 cores.

Sharding: core i = (batch b=i//2, head-group g=i%2 of 8 heads).
Each core runs the full pipeline for its (b, g) shard with zero
cross-core communication; the row-parallel out_proj partial sums of the
two head-groups of a batch are added on the host during unsharding.

Compute dtype: bfloat16 operands, fp32 PSUM accumulation (full PE rate).
Host converts inputs to bf16; the final output is fp32.

Per-core dataflow:
  phase 1: qkv projection from xT (feature-major x), producing
           qT/kT [512,2048] feature-major and v [2048,512] time-major,
           spilled to DRAM scratch (bf16).
  phase 2: per head: scores^T tiles [t2,t1] = kT_h' @ qT_h (K=64),
           exp on ScalarE (no max subtraction -- scores are O(1)),
           causal triangular mask on diagonal 128x128 blocks only,
           PV matmul with lhsT=[v_h | ones] so PSUM row 64 accumulates
           the softmax denominator; normalize on the way to SBUF.
  phase 3: out_proj partial = attn_outT' @ w_out rows for this group.
"""

import os
import sys

for _p in (
    "/root/.axon_site",
    "/root/.axon_site/_ro/trn_rl_repo",
    "/root/.axon_site/_ro/pypackages",
    "/opt/trn_rl_repo",
):
    if os.path.isdir(_p) and _p not in sys.path:
        sys.path.append(_p)

import numpy as np
import ml_dtypes

import concourse.bass as bass
import concourse.bacc as bacc
import concourse.mybir as mybir
from concourse import tile
from concourse.bass_utils import run_bass_kernel_spmd

BF16NP = ml_dtypes.bfloat16

B, T, C, H, D = 4, 2048, 1024, 16, 64
HPC = 8            # heads per core
GF = HPC * D       # 512: feature width of one head-group
NCORES = 8
KC = C // 128      # 8 contraction tiles over C
NT = T // 128      # 16 time tiles of 128
TS = 512           # t1 slice width
NTS = T // TS      # 4 t1 slices

F32 = mybir.dt.float32
BF16 = mybir.dt.bfloat16
AF = mybir.ActivationFunctionType


def build_nc() -> bass.Bass:
    nc = bacc.Bacc()

    xT = nc.declare_dram_parameter("xT", [C, T], BF16, isOutput=False)
    wq = nc.declare_dram_parameter("wq", [C, GF], BF16, isOutput=False)
    wk = nc.declare_dram_parameter("wk", [C, GF], BF16, isOutput=False)
    wv = nc.declare_dram_parameter("wv", [C, GF], BF16, isOutput=False)
    wo = nc.declare_dram_parameter("wo", [GF, C], BF16, isOutput=False)
    bqc = nc.declare_dram_parameter("bqc", [128, 4], F32, isOutput=False)
    bkc = nc.declare_dram_parameter("bkc", [128, 4], F32, isOutput=False)
    bv = nc.declare_dram_parameter("bv", [1, GF], BF16, isOutput=False)
    trineg = nc.declare_dram_parameter("trineg", [128, 128], F32, isOutput=False)
    out = nc.declare_dram_parameter("out", [T, C], F32, isOutput=True)

    with tile.TileContext(nc) as tc:
        with (
            tc.tile_pool(name="dram", bufs=1, space="DRAM") as dpool,
            tc.tile_pool(name="consts", bufs=1) as cpool,
            tc.tile_pool(name="apool", bufs=1) as apool,
        ):
            # v resident in SBUF: [t2-part, t2-tile, head, 64 v + 1 ones]
            v_sb = apool.tile([128, NT, HPC, 65], BF16, tag="vsb")
            nc.vector.memset(v_sb[:, :, :, 64:65], 1.0)
            # attention outputs stay resident in SBUF, feature-major
            aP = [apool.tile([128, T], BF16, tag=f"aP{f}", name=f"aP{f}") for f in range(4)]

            ones = cpool.tile([1, TS], BF16)
            nc.vector.memset(ones[:, :], 1.0)
            tn_sb = cpool.tile([128, 128], F32)
            nc.sync.dma_start(tn_sb[:, :], trineg[:, :])
            bqc_sb = cpool.tile([128, 4], F32, tag="bqc")
            nc.sync.dma_start(bqc_sb[:, :], bqc[:, :])
            bkc_sb = cpool.tile([128, 4], F32, tag="bkc")
            nc.sync.dma_start(bkc_sb[:, :], bkc[:, :])
            bv_sb = cpool.tile([1, GF], BF16, tag="bv")
            nc.sync.dma_start(bv_sb[:, :], bv[:, :])

            # ------- phases 1+2 interleaved: qkv projection + attention -------
            with (
                tc.tile_pool(name="p1", bufs=1) as p1,
                tc.tile_pool(name="p1s", bufs=16) as p1s,
                tc.tile_pool(name="p2", bufs=2) as p2,
                tc.tile_pool(name="pE", bufs=10) as pE,
                tc.tile_pool(name="p3s", bufs=6) as p3s,
                tc.tile_pool(name="pp1", bufs=2, space="PSUM") as pp1,
                tc.tile_pool(name="ps2", bufs=1, space="PSUM") as ps2,
                tc.tile_pool(name="pso", bufs=2, space="PSUM") as pso,
            ):
                wo_sb = p1.tile([128, 4, C], BF16, tag="wo")
                nc.sync.dma_start(
                    wo_sb[:, :, :], wo[:, :].rearrange("(n p) c -> p n c", p=128)
                )
                xT_sb = p1.tile([128, KC, T], BF16)
                wq_sb = p1.tile([128, KC, GF], BF16, tag="wq")
                wk_sb = p1.tile([128, KC, GF], BF16, tag="wk")
                wv_sb = p1.tile([128, KC, GF], BF16, tag="wv")
                for k in range(KC):
                    ksl = slice(k * 128, (k + 1) * 128)
                    nc.sync.dma_start(xT_sb[:, k, :], xT[ksl, :])
                    nc.sync.dma_start(wv_sb[:, k, :], wv[ksl, :])
                    nc.sync.dma_start(wq_sb[:, k, :], wq[ksl, :])
                    nc.sync.dma_start(wk_sb[:, k, :], wk[ksl, :])

                for f in range(4):
                    # project q/k feature tile f straight into resident SBUF
                    # pair tiles (rows 0-63 head 2f, rows 64-127 head 2f+1)
                    qp = p2.tile([128, T], BF16, tag="qp")
                    kp = p2.tile([128, T], BF16, tag="kp")
                    for w_sb, b_sb, dst in (
                        (wq_sb, bqc_sb, qp),
                        (wk_sb, bkc_sb, kp),
                    ):
                        for ts in range(NTS):
                            acc = pp1.tile([128, TS], F32, tag="acc")
                            for k in range(KC):
                                nc.tensor.matmul(
                                    acc[:, :],
                                    w_sb[:, k, f * 128 : (f + 1) * 128],
                                    xT_sb[:, k, ts * TS : (ts + 1) * TS],
                                    start=(k == 0),
                                    stop=(k == KC - 1),
                                )
                            nc.vector.tensor_scalar_add(
                                dst[:, ts * TS : (ts + 1) * TS],
                                acc[:, :],
                                b_sb[:, f : f + 1],
                            )

                    if f == 0:
                        # v first: every head needs all of v.
                        for t in range(NT):
                            acc = pp1.tile([128, GF], F32, tag="acc")
                            for k in range(KC):
                                nc.tensor.matmul(
                                    acc[:, :],
                                    xT_sb[:, k, t * 128 : (t + 1) * 128],
                                    wv_sb[:, k, :],
                                    start=(k == 0),
                                    stop=False,
                                )
                            nc.tensor.matmul(
                                acc[:, :], ones[:, 0:128], bv_sb[:, :], start=False, stop=True
                            )
                            nc.scalar.copy(
                                v_sb[:, t, :, 0:64],
                                acc[:, :].rearrange("p (h d) -> p h d", h=HPC),
                            )

                    for t1i in range(NTS):
                        t1s = t1i * TS
                        nfull = t1s // 128
                        o_ps = [
                            pso.tile([65, TS], F32, tag="outps", name=f"ops{r}")
                            for r in range(2)
                        ]
                        nmm = [0, 0]
                        batches = [("full", t2p) for t2p in range(nfull // 2)]
                        batches += [("diag", 0), ("diag", 1)]

                        def emit_scores(b, r):
                            kind, idx = b
                            s_ps = ps2.tile(
                                [128, 1024], F32, tag=f"sps{r}", name=f"sps{r}"
                            )
                            if kind == "full":
                                for j in range(2):
                                    t2 = 2 * idx + j
                                    nc.tensor.matmul(
                                        s_ps[:, j * TS : (j + 1) * TS],
                                        kp[r * 64 : (r + 1) * 64, t2 * 128 : (t2 + 1) * 128],
                                        qp[r * 64 : (r + 1) * 64, t1s : t1s + TS],
                                        start=True,
                                        stop=True,
                                    )
                            else:
                                ds = (0, 1) if idx == 0 else (2, 3)
                                offs = (0, TS) if idx == 0 else (0, 256)
                                for d, off in zip(ds, offs):
                                    t2 = nfull + d
                                    nd = TS - 128 * d
                                    nc.tensor.matmul(
                                        s_ps[:, off : off + nd],
                                        kp[r * 64 : (r + 1) * 64, t2 * 128 : (t2 + 1) * 128],
                                        qp[r * 64 : (r + 1) * 64, t1s + 128 * d : t1s + TS],
                                        start=True,
                                        stop=True,
                                    )
                                for d, off in zip(ds, offs):
                                    nc.vector.tensor_add(
                                        s_ps[:, off : off + 128],
                                        s_ps[:, off : off + 128],
                                        tn_sb[:, :],
                                    )
                            return s_ps

                        def emit_exp(b, r, s_ps):
                            kind, idx = b
                            E = pE.tile([128, 1024], BF16, tag="E", name=f"E{r}")
                            if kind == "full":
                                nc.scalar.activation(E[:, :], s_ps[:, :], AF.Exp)
                            else:
                                width = 896 if idx == 0 else 384
                                nc.scalar.activation(
                                    E[:, 0:width], s_ps[:, 0:width], AF.Exp
                                )
                            return E

                        def emit_pv(b, r, E):
                            kind, idx = b
                            if kind == "full":
                                for j in range(2):
                                    t2 = 2 * idx + j
                                    nc.tensor.matmul(
                                        o_ps[r][:, :],
                                        v_sb[:, t2, 2 * f + r, :],
                                        E[:, j * TS : (j + 1) * TS],
                                        start=(nmm[r] == 0),
                                        stop=False,
                                    )
                                    nmm[r] += 1
                            else:
                                ds = (0, 1) if idx == 0 else (2, 3)
                                offs = (0, TS) if idx == 0 else (0, 256)
                                for d, off in zip(ds, offs):
                                    t2 = nfull + d
                                    nd = TS - 128 * d
                                    nc.tensor.matmul(
                                        o_ps[r][:, 128 * d : TS],
                                        v_sb[:, t2, 2 * f + r, :],
                                        E[:, off : off + nd],
                                        start=(nmm[r] == 0),
                                        stop=(d == 3),
                                    )
                                    nmm[r] += 1

                        # software pipeline: scores(i+1) emitted between the
                        # two exps of batch i so ACT never waits behind PVs
                        sps_cur = [emit_scores(batches[0], r) for r in range(2)]
                        pvq = []
                        for i, b in enumerate(batches):
                            Es = [None, None]
                            sps_nxt = [None, None]
                            for r in range(2):
                                Es[r] = emit_exp(b, r, sps_cur[r])
                                if i + 1 < len(batches):
                                    sps_nxt[r] = emit_scores(batches[i + 1], r)
                            if len(pvq) >= 3:
                                pb, pEs = pvq.pop(0)
                                for r in range(2):
                                    emit_pv(pb, r, pEs[r])
                            pvq.append((b, Es))
                            sps_cur = sps_nxt
                        for pb, pEs in pvq:
                            for r in range(2):
                                emit_pv(pb, r, pEs[r])

                        # normalize rows 0-63 by row 64 (softmax denominator)
                        for r in range(2):
                            denom = pE.tile([1, TS], F32, tag="dn", name=f"dn{r}")
                            nc.vector.tensor_copy(denom[:, :], o_ps[r][64:65, :])
                            bc_sb = pE.tile([64, TS], F32, tag="bc", name=f"bc{r}")
                            nc.gpsimd.partition_broadcast(bc_sb[:, :], denom[:, :])
                            rc_sb = pE.tile([64, TS], F32, tag="rc", name=f"rc{r}")
                            nc.vector.reciprocal_approx_fast(rc_sb[:, :], bc_sb[:, :])
                            nc.vector.tensor_mul(
                                aP[f][r * 64 : (r + 1) * 64, t1s : t1s + TS],
                                o_ps[r][0:64, :],
                                rc_sb[:, :],
                            )
                        if f == 3:
                            # all heads done for this t1 range: output projection
                            for t in range(t1s // 128, t1s // 128 + 4):
                                for n in range(2):
                                    acc3 = pp1.tile([128, TS], F32, tag="acc")
                                    for ff in range(4):
                                        nc.tensor.matmul(
                                            acc3[:, :],
                                            aP[ff][:, t * 128 : (t + 1) * 128],
                                            wo_sb[:, ff, n * TS : (n + 1) * TS],
                                            start=(ff == 0),
                                            stop=(ff == 3),
                                        )
                                    stg = p3s.tile([128, TS], F32, tag="stg3")
                                    if (t + n) % 2 == 0:
                                        nc.vector.tensor_copy(stg[:, :], acc3[:, :])
                                    else:
                                        nc.scalar.copy(stg[:, :], acc3[:, :])
                                    nc.gpsimd.dma_start(
                                        out[
                                            t * 128 : (t + 1) * 128,
                                            n * TS : (n + 1) * TS,
                                        ],
                                        stg[:, :],
                                    )

    nc.finalize()
    return nc


def make_in_maps(x, w_qkv, b_qkv, w_out, b_out):
    x = np.asarray(x, dtype=np.float32)
    w_qkv = np.asarray(w_qkv, dtype=np.float32)
    b_qkv = np.asarray(b_qkv, dtype=np.float32)
    w_out = np.asarray(w_out, dtype=np.float32)

    def bf(a):
        return np.ascontiguousarray(a).astype(BF16NP)

    scale = 1.0 / np.sqrt(D)
    # additive causal mask for diagonal blocks: 0 where t1 >= t2, -30 where
    # t2 > t1 (exp(-30+smax) is negligible vs any denominator)
    trineg = np.where(
        np.triu(np.ones((128, 128), dtype=np.float32)) > 0, 0.0, -30.0
    ).astype(np.float32)
    in_maps = []
    for core in range(NCORES):
        b, g = core // 2, core % 2
        sl = slice(g * GF, (g + 1) * GF)
        bq = (b_qkv[sl] * scale).reshape(4, 128).T  # [128, 4] per-feat col
        bk = b_qkv[C + g * GF : C + (g + 1) * GF].reshape(4, 128).T
        in_maps.append(
            {
                "xT": bf(x[b].T),
                "wq": bf(w_qkv[:, sl] * scale),
                "wk": bf(w_qkv[:, C + g * GF : C + (g + 1) * GF]),
                "wv": bf(w_qkv[:, 2 * C + g * GF : 2 * C + (g + 1) * GF]),
                "wo": bf(w_out[sl, :]),
                "bqc": np.ascontiguousarray(bq, dtype=np.float32),
                "bkc": np.ascontiguousarray(bk, dtype=np.float32),
                "bv": bf(b_qkv[2 * C + g * GF : 2 * C + (g + 1) * GF].reshape(1, GF)),
                "trineg": trineg,
            }
        )
    return in_maps


_NC_CACHE = {}


def run(inputs: dict, trace: bool = False):
    """Compile (cached) + run on 8 cores. Returns (full_output, BassKernelResults)."""
    if "nc" not in _NC_CACHE:
        _NC_CACHE["nc"] = build_nc()
    nc = _NC_CACHE["nc"]
    in_maps = make_in_maps(**inputs)
    res = run_bass_kernel_spmd(
        nc, in_maps, core_ids=list(range(NCORES)), trace=trace
    )
    outs = [np.asarray(m["out"], dtype=np.float32) for m in res.results]
    full = np.stack([outs[2 * b] + outs[2 * b + 1] for b in range(B)], axis=0)
    full += np.asarray(inputs["b_out"], dtype=np.float32)
    return full, res


def kernel(**inputs) -> np.ndarray:
    full, _ = run(inputs, trace=False)
    return full

